# revision 15
# baseline (speedup 1.0000x reference)
"""Trainium2 Bass kernel for nn_MultiHeadAttention_89154931130794.

Data-parallel over batch: 8 batch elements -> 8 NeuronCores, one full
attention block per core (no collectives needed; per-batch outputs are
independent).

Per-core pipeline (S=2048, E=1024, H=8, D=128):
  phase 0: load + transpose weights (bf16) via SWDGE cast-DMA + DMA-xbar
  phase 1: xT = x^T (bf16), QT/KT (head-dim major) + V (token major)
  phase 2: per (q-block, head): scores on PE -> exp on ScalarE (fused
           row-sum accum) -> normalize on VectorE -> head-mean accum;
           DMA-xbar transpose of attn -> ctx^T on PE -> out-proj on PE
           -> residual + LayerNorm -> token-sum via PE ones-matmul.
Outputs: attn_weights (mean over heads) [2048,2048] f32, context row
(mean over tokens of LN output) [1,1024] f32.
"""

import math

import numpy as np

B, S, E, H, D = 8, 2048, 1024, 8, 128
ST = S // 128   # token chunks
ET = E // 128   # embed chunks
NG = S // 512   # 512-wide column groups of the score matrix
QG = 4          # q-block groups (4 q-blocks of 128 each)
QBG = 4
LN_EPS = 1e-5
SCALE = 1.0 / math.sqrt(D)

_CACHE = {}


def _build():
    import concourse.bass as bass
    import concourse.tile as tile
    from concourse import bacc, mybir

    f32 = mybir.dt.float32
    bf16 = mybir.dt.bfloat16
    AF = mybir.ActivationFunctionType
    OP = mybir.AluOpType

    nc = bacc.Bacc("TRN2", target_bir_lowering=False, debug=False, num_devices=8)

    x_d = nc.declare_dram_parameter("x", [S, E], f32, isOutput=False)
    wqkv_d = nc.declare_dram_parameter("w_qkv", [3 * E, E], f32, isOutput=False)
    bqkv_d = nc.declare_dram_parameter("b_qkv", [3 * E], f32, isOutput=False)
    wout_d = nc.declare_dram_parameter("w_out", [E, E], f32, isOutput=False)
    bout_d = nc.declare_dram_parameter("b_out", [E], f32, isOutput=False)
    g_d = nc.declare_dram_parameter("ln_g", [E], f32, isOutput=False)
    bln_d = nc.declare_dram_parameter("ln_b", [E], f32, isOutput=False)

    attn_d = nc.declare_dram_parameter("attn_w", [S, S], f32, isOutput=True)
    ctx_d = nc.declare_dram_parameter("ctx_out", [1, E], f32, isOutput=True)

    with tile.TileContext(nc) as tc:
        with (
            tc.tile_pool(name="const", bufs=1) as cpool,
            tc.tile_pool(name="persist", bufs=1) as ppool,
        ):
            # ---- constants ----
            eps_t = cpool.tile([128, 1], f32)
            nc.vector.memset(eps_t, LN_EPS)
            ones_row_bf = cpool.tile([1, 128], bf16)
            nc.vector.memset(ones_row_bf, 1.0)
            ones_col_f32 = cpool.tile([128, 1], f32)
            nc.vector.memset(ones_col_f32, 1.0)
            outb_row_bf = cpool.tile([1, E], bf16)
            nc.gpsimd.dma_start(
                out=outb_row_bf, in_=bout_d.ap().rearrange("(a o) -> a o", a=1)
            )

            # ---- persistent big tensors ----
            OWT = ppool.tile([128, ET, E], bf16)      # [i_part, ic, o]
            QT = ppool.tile([128, H, S], bf16)        # [d, h, tok]
            KT = ppool.tile([128, H, S], bf16)        # [d, h, tok]
            V = ppool.tile([128, H, ST, 128], bf16)   # [tok_part, h, kc, d]
            S3_sb = ppool.tile([1, E], f32)           # sum over tokens of LN out

            # ================= phase 0/1: weights, xT, Q/K/V =================
            with (
                tc.tile_pool(name="ph1", bufs=3) as st1,
                tc.tile_pool(name="ph1w", bufs=2) as stw,
                tc.tile_pool(name="pp1", bufs=4, space="PSUM") as pp1,
            ):
                # per-(head, q/k/v) bias columns: bias_sb[p, c] = b_qkv[c*128 + p]
                bias_sb = st1.tile([128, 3 * ET], f32, bufs=1)
                nc.sync.dma_start(
                    out=bias_sb, in_=bqkv_d.ap().rearrange("(c p) -> p c", p=128)
                )
                # alternate HWDGE rings (SP / ACT) so transposes overlap
                rings = (nc.sync, nc.scalar)

                # out_w -> OWT (transposed, bf16)
                for r in range(ET):
                    ow_bf = st1.tile([128, E], bf16, tag="stage")
                    nc.gpsimd.dma_start(
                        out=ow_bf, in_=wout_d.ap()[r * 128 : (r + 1) * 128, :]
                    )
                    rings[r % 2].dma_start(
                        out=OWT[:, :, r * 128 : (r + 1) * 128], in_=ow_bf,
                        transpose=True,
                    )

                # x -> xT (transposed, bf16): xT[e_part, ec, t]
                xT = st1.tile([128, ET, S], bf16, tag="xT", bufs=1)
                for t in range(ST):
                    x_bf = st1.tile([128, E], bf16, tag="stage")
                    nc.gpsimd.dma_start(
                        out=x_bf, in_=x_d.ap()[t * 128 : (t + 1) * 128, :]
                    )
                    rings[t % 2].dma_start(
                        out=xT[:, :, t * 128 : (t + 1) * 128], in_=x_bf,
                        transpose=True,
                    )

                for h in range(H):
                    # transpose the three weight row-blocks of this head
                    wts = []
                    for m in range(3):  # q, k, v
                        w_bf = st1.tile([128, E], bf16, tag="stage")
                        r0 = m * E + h * 128
                        nc.gpsimd.dma_start(out=w_bf, in_=wqkv_d.ap()[r0 : r0 + 128, :])
                        wt = stw.tile([128, ET, 128], bf16, tag=f"wt{m}")
                        rings[m % 2].dma_start(out=wt, in_=w_bf, transpose=True)
                        wts.append(wt)
                    wtq, wtk, wtv = wts

                    # QT_h / KT_h / VT_h  [d, tok]; bias added during the
                    # PSUM->SBUF copy; V is then re-transposed to token-major.
                    vt = stw.tile([128, S], bf16, tag="vt")
                    for which, wt, bc in (
                        (QT[:, h, :], wtq, h),
                        (KT[:, h, :], wtk, ET + h),
                        (vt, wtv, 2 * ET + h),
                    ):
                        for ng in range(NG):
                            ps = pp1.tile([128, 512], f32, tag="qk")
                            for ec in range(ET):
                                nc.tensor.matmul(
                                    ps,
                                    lhsT=wt[:, ec, :],
                                    rhs=xT[:, ec, ng * 512 : (ng + 1) * 512],
                                    start=(ec == 0),
                                    stop=(ec == ET - 1),
                                )
                            nc.vector.tensor_scalar(
                                out=which[:, ng * 512 : (ng + 1) * 512],
                                in0=ps,
                                scalar1=bias_sb[:, bc : bc + 1],
                                scalar2=None,
                                op0=OP.add,
                            )
                    rings[h % 2].dma_start(
                        out=V[:, h, :, :], in_=vt, transpose=True
                    )

            # ======================= phase 2: attention =======================
            with (
                tc.tile_pool(name="acc", bufs=4) as apool,
                tc.tile_pool(name="exps", bufs=2) as epool,
                tc.tile_pool(name="attnT", bufs=2) as tpool,
                tc.tile_pool(name="ct", bufs=1) as ctpool,
                tc.tile_pool(name="small", bufs=4) as spool,
                tc.tile_pool(name="post", bufs=2) as opool,
                tc.tile_pool(name="ps_s", bufs=2, space="PSUM") as ps_s,
                tc.tile_pool(name="ps_c", bufs=2, space="PSUM") as ps_c,
                tc.tile_pool(name="ps_y", bufs=1, space="PSUM") as ps_y,
            ):
                for qg in range(QG):
                    acc_tiles = [
                        apool.tile([128, S], bf16, tag="acc", name=f"acc{qg}_{i}")
                        for i in range(QBG)
                    ]
                    CT = ctpool.tile([128, H, 512], bf16, tag="ct")
                    for h in range(H):
                        attnT = tpool.tile([128, ST, 512], bf16, tag="attnT")
                        for qb in range(QBG):
                            qbg = qg * QBG + qb
                            exps = epool.tile([128, S], bf16, tag="exps")
                            rs2 = spool.tile([128, 2], f32, tag="rs2")
                            for half in range(2):
                                pss = ps_s.tile([128, 1024], f32, tag="s")
                                for n2 in range(2):
                                    ng = half * 2 + n2
                                    nc.tensor.matmul(
                                        pss[:, n2 * 512 : (n2 + 1) * 512],
                                        lhsT=QT[:, h, qbg * 128 : qbg * 128 + 128],
                                        rhs=KT[:, h, ng * 512 : (ng + 1) * 512],
                                        start=True,
                                        stop=True,
                                    )
                                nc.scalar.activation(
                                    out=exps[:, half * 1024 : (half + 1) * 1024],
                                    in_=pss,
                                    func=AF.Exp,
                                    scale=SCALE,
                                    accum_out=rs2[:, half : half + 1],
                                )
                            rs = spool.tile([128, 1], f32, tag="rs")
                            nc.vector.tensor_add(
                                out=rs, in0=rs2[:, 0:1], in1=rs2[:, 1:2]
                            )
                            r8 = spool.tile([128, 1], f32, tag="r8")
                            nc.vector.reciprocal(out=r8, in_=rs)
                            # exps <- attn (normalized), in place
                            nc.vector.tensor_scalar(
                                out=exps, in0=exps, scalar1=r8, scalar2=None,
                                op0=OP.mult,
                            )
                            # head-mean accumulation: acc += attn/8
                            if h == 0:
                                nc.vector.tensor_scalar(
                                    out=acc_tiles[qb], in0=exps, scalar1=0.125,
                                    scalar2=None, op0=OP.mult,
                                )
                            else:
                                nc.vector.scalar_tensor_tensor(
                                    out=acc_tiles[qb],
                                    in0=exps,
                                    scalar=0.125,
                                    in1=acc_tiles[qb],
                                    op0=OP.mult,
                                    op1=OP.add,
                                )
                            rings[(h + qb) % 2].dma_start(
                                out=attnT[:, :, qb * 128 : (qb + 1) * 128],
                                in_=exps,
                                transpose=True,
                            )
                        # ctx^T for this head over the 512 queries of the group
                        psc = ps_c.tile([128, 512], f32, tag="c")
                        for kc in range(ST):
                            nc.tensor.matmul(
                                psc,
                                lhsT=V[:, h, kc, :],
                                rhs=attnT[:, kc, :],
                                start=(kc == 0),
                                stop=(kc == ST - 1),
                            )
                        nc.scalar.activation(
                            out=CT[:, h, :], in_=psc, func=AF.Copy
                        )

                    # ---- out-proj + residual + LayerNorm per q-block ----
                    for qb in range(QBG):
                        qbg = qg * QBG + qb
                        psy = ps_y.tile([128, E], f32, tag="y")
                        for oc in range(2):
                            for ic in range(ET):
                                nc.tensor.matmul(
                                    psy[:, oc * 512 : (oc + 1) * 512],
                                    lhsT=CT[:, ic, qb * 128 : (qb + 1) * 128],
                                    rhs=OWT[:, ic, oc * 512 : (oc + 1) * 512],
                                    start=(ic == 0),
                                    stop=False,
                                )
                            nc.tensor.matmul(
                                psy[:, oc * 512 : (oc + 1) * 512],
                                lhsT=ones_row_bf,
                                rhs=outb_row_bf[:, oc * 512 : (oc + 1) * 512],
                                start=False,
                                stop=True,
                            )
                        x_f = opool.tile([128, E], f32, tag="xr")
                        rings[qb % 2].dma_start(
                            out=x_f, in_=x_d.ap()[qbg * 128 : qbg * 128 + 128, :]
                        )
                        t1 = opool.tile([128, E], f32, tag="t1")
                        nc.vector.tensor_add(out=t1, in0=psy, in1=x_f)
                        stats = spool.tile([128, 2, 6], f32, tag="stats")
                        for sg in range(2):
                            nc.vector.bn_stats(
                                out=stats[:, sg, :],
                                in_=t1[:, sg * 512 : (sg + 1) * 512],
                            )
                        mv = spool.tile([128, 2], f32, tag="mv")
                        nc.vector.bn_aggr(out=mv, in_=stats)
                        sd = spool.tile([128, 1], f32, tag="sd")
                        nc.scalar.activation(
                            out=sd, in_=mv[:, 1:2], func=AF.Sqrt, bias=eps_t
                        )
                        rstd = spool.tile([128, 1], f32, tag="rstd")
                        nc.vector.reciprocal(out=rstd, in_=sd)
                        # t1 <- (t1 - mean) * rstd, in place
                        nc.vector.tensor_scalar(
                            out=t1, in0=t1, scalar1=mv[:, 0:1], scalar2=rstd,
                            op0=OP.subtract, op1=OP.mult,
                        )
                        # token-sum of LN output (g/b applied at the end)
                        ps3 = ps_y.tile([1, E], f32, tag="y")
                        for oc in range(2):
                            nc.tensor.matmul(
                                ps3[:, oc * 512 : (oc + 1) * 512],
                                lhsT=ones_col_f32,
                                rhs=t1[:, oc * 512 : (oc + 1) * 512],
                                start=True,
                                stop=True,
                            )
                        if qbg == 0:
                            nc.vector.tensor_copy(out=S3_sb, in_=ps3)
                        else:
                            nc.vector.tensor_add(out=S3_sb, in0=S3_sb, in1=ps3)

                    # stream the head-mean out (bf16 -> f32 cast in DMA)
                    for qb in range(QBG):
                        qbg = qg * QBG + qb
                        nc.gpsimd.dma_start(
                            out=attn_d.ap()[qbg * 128 : qbg * 128 + 128, :],
                            in_=acc_tiles[qb],
                        )

                # ---- context row ----
                g_row = opool.tile([1, E], f32, tag="xr")
                nc.sync.dma_start(
                    out=g_row, in_=g_d.ap().rearrange("(a o) -> a o", a=1)
                )
                bln_row = opool.tile([1, E], f32, tag="xr")
                nc.sync.dma_start(
                    out=bln_row, in_=bln_d.ap().rearrange("(a o) -> a o", a=1)
                )
                ctx_row = spool.tile([1, E], f32, tag="ctx", bufs=1)
                nc.vector.tensor_scalar(
                    out=ctx_row, in0=S3_sb, scalar1=1.0 / S, scalar2=None,
                    op0=OP.mult,
                )
                nc.vector.tensor_mul(out=ctx_row, in0=ctx_row, in1=g_row)
                nc.vector.tensor_add(out=ctx_row, in0=ctx_row, in1=bln_row)
                nc.sync.dma_start(out=ctx_d.ap(), in_=ctx_row)

    nc.finalize()
    return nc


def _get_nc():
    if "nc" not in _CACHE:
        _CACHE["nc"] = _build()
    return _CACHE["nc"]


def kernel(x, in_proj_w, in_proj_b, out_w, out_b, ln_g, ln_b):
    from concourse.bass_utils import run_bass_kernel_spmd

    nc = _get_nc()
    shared = {
        "w_qkv": np.ascontiguousarray(in_proj_w, dtype=np.float32),
        "b_qkv": np.ascontiguousarray(in_proj_b, dtype=np.float32),
        "w_out": np.ascontiguousarray(out_w, dtype=np.float32),
        "b_out": np.ascontiguousarray(out_b, dtype=np.float32),
        "ln_g": np.ascontiguousarray(ln_g, dtype=np.float32),
        "ln_b": np.ascontiguousarray(ln_b, dtype=np.float32),
    }
    in_maps = [
        {"x": np.ascontiguousarray(x[b], dtype=np.float32), **shared}
        for b in range(B)
    ]
    res = run_bass_kernel_spmd(nc, in_maps, core_ids=list(range(B)))
    context = np.stack([res.results[b]["ctx_out"][0] for b in range(B)])
    attn_weights = np.stack([res.results[b]["attn_w"] for b in range(B)])
    return (context, attn_weights)


# revision 16
# speedup vs baseline: 1.0580x; 1.0580x over previous
"""Trainium2 Bass kernel for nn_MultiHeadAttention_89154931130794.

Data-parallel over batch: 8 batch elements -> 8 NeuronCores, one full
attention block per core (no collectives needed; per-batch outputs are
independent).

Per-core pipeline (S=2048, E=1024, H=8, D=128):
  phase 0: load + transpose weights (bf16) via SWDGE cast-DMA + DMA-xbar
  phase 1: xT = x^T (bf16), QT/KT (head-dim major) + V (token major)
  phase 2: per (q-block, head): scores on PE -> exp on ScalarE (fused
           row-sum accum) -> normalize on VectorE -> head-mean accum;
           DMA-xbar transpose of attn -> ctx^T on PE -> out-proj on PE
           -> residual + LayerNorm -> token-sum via PE ones-matmul.
Outputs: attn_weights (mean over heads) [2048,2048] f32, context row
(mean over tokens of LN output) [1,1024] f32.
"""

import math

import numpy as np

B, S, E, H, D = 8, 2048, 1024, 8, 128
ST = S // 128   # token chunks
ET = E // 128   # embed chunks
NG = S // 512   # 512-wide column groups of the score matrix
QG = 4          # q-block groups (4 q-blocks of 128 each)
QBG = 4
LN_EPS = 1e-5
SCALE = 1.0 / math.sqrt(D)

_CACHE = {}


def _build():
    import concourse.bass as bass
    import concourse.tile as tile
    from concourse import bacc, mybir

    f32 = mybir.dt.float32
    bf16 = mybir.dt.bfloat16
    AF = mybir.ActivationFunctionType
    OP = mybir.AluOpType

    nc = bacc.Bacc("TRN2", target_bir_lowering=False, debug=False, num_devices=8)

    x_d = nc.declare_dram_parameter("x", [S, E], f32, isOutput=False)
    wqkv_d = nc.declare_dram_parameter("w_qkv", [3 * E, E], f32, isOutput=False)
    bqkv_d = nc.declare_dram_parameter("b_qkv", [3 * E], f32, isOutput=False)
    wout_d = nc.declare_dram_parameter("w_out", [E, E], f32, isOutput=False)
    bout_d = nc.declare_dram_parameter("b_out", [E], f32, isOutput=False)
    g_d = nc.declare_dram_parameter("ln_g", [E], f32, isOutput=False)
    bln_d = nc.declare_dram_parameter("ln_b", [E], f32, isOutput=False)

    attn_d = nc.declare_dram_parameter("attn_w", [S, S], f32, isOutput=True)
    ctx_d = nc.declare_dram_parameter("ctx_out", [1, E], f32, isOutput=True)

    with tile.TileContext(nc) as tc:
        with (
            tc.tile_pool(name="const", bufs=1) as cpool,
            tc.tile_pool(name="persist", bufs=1) as ppool,
        ):
            # ---- constants ----
            eps_t = cpool.tile([128, 1], f32)
            nc.vector.memset(eps_t, LN_EPS)
            ones_row_bf = cpool.tile([1, 128], bf16)
            nc.vector.memset(ones_row_bf, 1.0)
            ones_col_f32 = cpool.tile([128, 1], f32)
            nc.vector.memset(ones_col_f32, 1.0)
            outb_row_bf = cpool.tile([1, E], bf16)
            nc.gpsimd.dma_start(
                out=outb_row_bf, in_=bout_d.ap().rearrange("(a o) -> a o", a=1)
            )

            # ---- persistent big tensors ----
            OWT = ppool.tile([128, ET, E], bf16)      # [i_part, ic, o]
            QT = ppool.tile([128, H, S], bf16)        # [d, h, tok]
            KT = ppool.tile([128, H, S], bf16)        # [d, h, tok]
            V = ppool.tile([128, H, ST, 128], bf16)   # [tok_part, h, kc, d]
            S3_sb = ppool.tile([1, E], f32)           # sum over tokens of LN out

            # ================= phase 0/1: weights, xT, Q/K/V =================
            with (
                tc.tile_pool(name="ph1", bufs=3) as st1,
                tc.tile_pool(name="ph1w", bufs=2) as stw,
                tc.tile_pool(name="pp1", bufs=4, space="PSUM") as pp1,
            ):
                # per-(head, q/k/v) bias columns: bias_sb[p, c] = b_qkv[c*128 + p]
                bias_sb = st1.tile([128, 3 * ET], f32, bufs=1)
                nc.gpsimd.dma_start(
                    out=bias_sb, in_=bqkv_d.ap().rearrange("(c p) -> p c", p=128)
                )
                # alternate HWDGE rings (SP / ACT) so transposes overlap
                rings = (nc.sync, nc.sync)

                # out_w -> OWT (transposed, bf16)
                for r in range(ET):
                    ow_bf = st1.tile([128, E], bf16, tag="stage")
                    nc.gpsimd.dma_start(
                        out=ow_bf, in_=wout_d.ap()[r * 128 : (r + 1) * 128, :]
                    )
                    rings[r % 2].dma_start(
                        out=OWT[:, :, r * 128 : (r + 1) * 128], in_=ow_bf,
                        transpose=True,
                    )

                # x -> xT (transposed, bf16): xT[e_part, ec, t]
                xT = st1.tile([128, ET, S], bf16, tag="xT", bufs=1)
                for t in range(ST):
                    x_bf = st1.tile([128, E], bf16, tag="stage")
                    nc.gpsimd.dma_start(
                        out=x_bf, in_=x_d.ap()[t * 128 : (t + 1) * 128, :]
                    )
                    rings[t % 2].dma_start(
                        out=xT[:, :, t * 128 : (t + 1) * 128], in_=x_bf,
                        transpose=True,
                    )

                for h in range(H):
                    # transpose the three weight row-blocks of this head
                    wts = []
                    for m in range(3):  # q, k, v
                        w_bf = st1.tile([128, E], bf16, tag="stage")
                        r0 = m * E + h * 128
                        nc.gpsimd.dma_start(out=w_bf, in_=wqkv_d.ap()[r0 : r0 + 128, :])
                        wt = stw.tile([128, ET, 128], bf16, tag=f"wt{m}")
                        rings[m % 2].dma_start(out=wt, in_=w_bf, transpose=True)
                        wts.append(wt)
                    wtq, wtk, wtv = wts

                    # QT_h / KT_h / VT_h  [d, tok]; bias added during the
                    # PSUM->SBUF copy; V is then re-transposed to token-major.
                    vt = stw.tile([128, S], bf16, tag="vt")
                    for which, wt, bc in (
                        (QT[:, h, :], wtq, h),
                        (KT[:, h, :], wtk, ET + h),
                        (vt, wtv, 2 * ET + h),
                    ):
                        for ng in range(NG):
                            ps = pp1.tile([128, 512], f32, tag="qk")
                            for ec in range(ET):
                                nc.tensor.matmul(
                                    ps,
                                    lhsT=wt[:, ec, :],
                                    rhs=xT[:, ec, ng * 512 : (ng + 1) * 512],
                                    start=(ec == 0),
                                    stop=(ec == ET - 1),
                                )
                            nc.vector.tensor_scalar(
                                out=which[:, ng * 512 : (ng + 1) * 512],
                                in0=ps,
                                scalar1=bias_sb[:, bc : bc + 1],
                                scalar2=None,
                                op0=OP.add,
                            )
                    rings[h % 2].dma_start(
                        out=V[:, h, :, :], in_=vt, transpose=True
                    )

            # ======================= phase 2: attention =======================
            with (
                tc.tile_pool(name="acc", bufs=4) as apool,
                tc.tile_pool(name="exps", bufs=3) as epool,
                tc.tile_pool(name="attnT", bufs=2) as tpool,
                tc.tile_pool(name="ct", bufs=1) as ctpool,
                tc.tile_pool(name="small", bufs=4) as spool,
                tc.tile_pool(name="post", bufs=2) as opool,
                tc.tile_pool(name="ps_s", bufs=2, space="PSUM") as ps_s,
                tc.tile_pool(name="ps_c", bufs=2, space="PSUM") as ps_c,
                tc.tile_pool(name="ps_y", bufs=1, space="PSUM") as ps_y,
            ):
                for qg in range(QG):
                    acc_tiles = [
                        apool.tile([128, S], bf16, tag="acc", name=f"acc{qg}_{i}")
                        for i in range(QBG)
                    ]
                    CT = ctpool.tile([128, H, 512], bf16, tag="ct")
                    for h in range(H):
                        attnT = tpool.tile([128, ST, 512], bf16, tag="attnT")
                        for qb in range(QBG):
                            qbg = qg * QBG + qb
                            exps = epool.tile([128, S], bf16, tag="exps")
                            rs2 = spool.tile([128, 2], f32, tag="rs2")
                            for half in range(2):
                                pss = ps_s.tile([128, 1024], f32, tag="s")
                                for n2 in range(2):
                                    ng = half * 2 + n2
                                    nc.tensor.matmul(
                                        pss[:, n2 * 512 : (n2 + 1) * 512],
                                        lhsT=QT[:, h, qbg * 128 : qbg * 128 + 128],
                                        rhs=KT[:, h, ng * 512 : (ng + 1) * 512],
                                        start=True,
                                        stop=True,
                                    )
                                nc.scalar.activation(
                                    out=exps[:, half * 1024 : (half + 1) * 1024],
                                    in_=pss,
                                    func=AF.Exp,
                                    scale=SCALE,
                                    accum_out=rs2[:, half : half + 1],
                                )
                            rs = spool.tile([128, 1], f32, tag="rs")
                            nc.vector.tensor_add(
                                out=rs, in0=rs2[:, 0:1], in1=rs2[:, 1:2]
                            )
                            r8 = spool.tile([128, 1], f32, tag="r8")
                            nc.vector.reciprocal(out=r8, in_=rs)
                            # exps <- attn (normalized), in place
                            nc.vector.tensor_scalar(
                                out=exps, in0=exps, scalar1=r8, scalar2=None,
                                op0=OP.mult,
                            )
                            # head-mean accumulation: acc += attn/8
                            if h == 0:
                                nc.vector.tensor_scalar(
                                    out=acc_tiles[qb], in0=exps, scalar1=0.125,
                                    scalar2=None, op0=OP.mult,
                                )
                            else:
                                nc.vector.scalar_tensor_tensor(
                                    out=acc_tiles[qb],
                                    in0=exps,
                                    scalar=0.125,
                                    in1=acc_tiles[qb],
                                    op0=OP.mult,
                                    op1=OP.add,
                                )
                            rings[(h + qb) % 2].dma_start(
                                out=attnT[:, :, qb * 128 : (qb + 1) * 128],
                                in_=exps,
                                transpose=True,
                            )
                        # ctx^T for this head over the 512 queries of the group
                        psc = ps_c.tile([128, 512], f32, tag="c")
                        for kc in range(ST):
                            nc.tensor.matmul(
                                psc,
                                lhsT=V[:, h, kc, :],
                                rhs=attnT[:, kc, :],
                                start=(kc == 0),
                                stop=(kc == ST - 1),
                            )
                        nc.scalar.activation(
                            out=CT[:, h, :], in_=psc, func=AF.Copy
                        )

                    # ---- out-proj + residual + LayerNorm per q-block ----
                    for qb in range(QBG):
                        qbg = qg * QBG + qb
                        psy = ps_y.tile([128, E], f32, tag="y")
                        for oc in range(2):
                            for ic in range(ET):
                                nc.tensor.matmul(
                                    psy[:, oc * 512 : (oc + 1) * 512],
                                    lhsT=CT[:, ic, qb * 128 : (qb + 1) * 128],
                                    rhs=OWT[:, ic, oc * 512 : (oc + 1) * 512],
                                    start=(ic == 0),
                                    stop=False,
                                )
                            nc.tensor.matmul(
                                psy[:, oc * 512 : (oc + 1) * 512],
                                lhsT=ones_row_bf,
                                rhs=outb_row_bf[:, oc * 512 : (oc + 1) * 512],
                                start=False,
                                stop=True,
                            )
                        x_f = opool.tile([128, E], f32, tag="xr")
                        nc.gpsimd.dma_start(
                            out=x_f, in_=x_d.ap()[qbg * 128 : qbg * 128 + 128, :]
                        )
                        t1 = opool.tile([128, E], f32, tag="t1")
                        nc.vector.tensor_add(out=t1, in0=psy, in1=x_f)
                        stats = spool.tile([128, 2, 6], f32, tag="stats")
                        for sg in range(2):
                            nc.vector.bn_stats(
                                out=stats[:, sg, :],
                                in_=t1[:, sg * 512 : (sg + 1) * 512],
                            )
                        mv = spool.tile([128, 2], f32, tag="mv")
                        nc.vector.bn_aggr(out=mv, in_=stats)
                        sd = spool.tile([128, 1], f32, tag="sd")
                        nc.scalar.activation(
                            out=sd, in_=mv[:, 1:2], func=AF.Sqrt, bias=eps_t
                        )
                        rstd = spool.tile([128, 1], f32, tag="rstd")
                        nc.vector.reciprocal(out=rstd, in_=sd)
                        # t1 <- (t1 - mean) * rstd, in place
                        nc.vector.tensor_scalar(
                            out=t1, in0=t1, scalar1=mv[:, 0:1], scalar2=rstd,
                            op0=OP.subtract, op1=OP.mult,
                        )
                        # token-sum of LN output (g/b applied at the end)
                        ps3 = ps_y.tile([1, E], f32, tag="y")
                        for oc in range(2):
                            nc.tensor.matmul(
                                ps3[:, oc * 512 : (oc + 1) * 512],
                                lhsT=ones_col_f32,
                                rhs=t1[:, oc * 512 : (oc + 1) * 512],
                                start=True,
                                stop=True,
                            )
                        if qbg == 0:
                            nc.vector.tensor_copy(out=S3_sb, in_=ps3)
                        else:
                            nc.vector.tensor_add(out=S3_sb, in0=S3_sb, in1=ps3)

                    # stream the head-mean out (bf16 -> f32 cast in DMA)
                    for qb in range(QBG):
                        qbg = qg * QBG + qb
                        nc.gpsimd.dma_start(
                            out=attn_d.ap()[qbg * 128 : qbg * 128 + 128, :],
                            in_=acc_tiles[qb],
                        )

                # ---- context row ----
                g_row = opool.tile([1, E], f32, tag="xr")
                nc.gpsimd.dma_start(
                    out=g_row, in_=g_d.ap().rearrange("(a o) -> a o", a=1)
                )
                bln_row = opool.tile([1, E], f32, tag="xr")
                nc.gpsimd.dma_start(
                    out=bln_row, in_=bln_d.ap().rearrange("(a o) -> a o", a=1)
                )
                ctx_row = spool.tile([1, E], f32, tag="ctx", bufs=1)
                nc.vector.tensor_scalar(
                    out=ctx_row, in0=S3_sb, scalar1=1.0 / S, scalar2=None,
                    op0=OP.mult,
                )
                nc.vector.tensor_mul(out=ctx_row, in0=ctx_row, in1=g_row)
                nc.vector.tensor_add(out=ctx_row, in0=ctx_row, in1=bln_row)
                nc.sync.dma_start(out=ctx_d.ap(), in_=ctx_row)

    nc.finalize()
    return nc


def _get_nc():
    if "nc" not in _CACHE:
        _CACHE["nc"] = _build()
    return _CACHE["nc"]


def kernel(x, in_proj_w, in_proj_b, out_w, out_b, ln_g, ln_b):
    from concourse.bass_utils import run_bass_kernel_spmd

    nc = _get_nc()
    shared = {
        "w_qkv": np.ascontiguousarray(in_proj_w, dtype=np.float32),
        "b_qkv": np.ascontiguousarray(in_proj_b, dtype=np.float32),
        "w_out": np.ascontiguousarray(out_w, dtype=np.float32),
        "b_out": np.ascontiguousarray(out_b, dtype=np.float32),
        "ln_g": np.ascontiguousarray(ln_g, dtype=np.float32),
        "ln_b": np.ascontiguousarray(ln_b, dtype=np.float32),
    }
    in_maps = [
        {"x": np.ascontiguousarray(x[b], dtype=np.float32), **shared}
        for b in range(B)
    ]
    res = run_bass_kernel_spmd(nc, in_maps, core_ids=list(range(B)))
    context = np.stack([res.results[b]["ctx_out"][0] for b in range(B)])
    attn_weights = np.stack([res.results[b]["attn_w"] for b in range(B)])
    return (context, attn_weights)
